# revision 24
# baseline (speedup 1.0000x reference)
"""Trainium2 Bass kernel for nn_HVGuardModel (dense MoE routing).

Reference math (B=65536, D=1024, E=8, H=128, C1=64, NC=2):
    gw  = softmax(x @ Wg + bg)                      [B, E]
    h   = relu(einsum('bd,edh', x, We1) + be1)      [B, E, H]
    eo  = einsum('beh,eho', h, We2) + be2           [B, E, H]
    mix = einsum('be,beh', gw, eo)                  [B, H]
    out = relu(mix @ Wc1 + bc1) @ Wc2 + bc2         [B, NC]

Strategy: pure data-parallel over 8 cores (8192 rows each).  All device
activations live in "feature-major" layout [feature, batch]; the host
supplies x pre-transposed (xT) and transposes the [2, 8192] per-core
outputs back.

Algebraic folds (host side):
  * mix is only consumed via mix @ Wc1  =>  fold V = We2 @ Wc1 per expert
    ([E*H, 64] stacked) and C = be2 @ Wc1; eo and mix are never materialized.
  * Layer-1 features are INTERLEAVED: f = j*E + e.  A "replicated gate"
    weight block (Wg columns tiled mod 8) yields a [128, N] logit tile whose
    row r holds logit[r mod 8], so no cross-partition broadcast is needed.
  * softmax denominator: all-ones [8,128] lhsT matmul replicates
    s = sum_e exp across all 128 partitions; 1/s on DVE.
  * All biases are per-partition -> ACT's native bias operand.

v2 over the fp32r baseline (373 us):
  * bf16 matmul operands everywhere.  Cost-model cycles/row are identical,
    but on HW fp32r matmuls must self-load their stationary operand (bass
    refuses standalone fp32r ldweights), serializing the weight load with
    the stream; bf16 gets a separate LDWEIGHTS the PE pulls ahead.  Also
    halves x DMA traffic (33.5 -> 16.8 MB/core) and doubles DVE throughput.
  * Software-pipelined PE program:  per tile t the PE order is
        gate(t), layer1(t), denom(t), Vfold(t-1), cls2(t-2)
    so every matmul's ACT/DVE-produced operands (exp -> 1/s -> gw -> hs)
    are computed a full tile (~18 us) before the PE consumes them; the
    softmax chain never stalls the PE.
  * x tiles prefetched 2 deep (xpool bufs=3).

Per 512-column batch tile: 83 matmuls = 42,496 PE columns -> 17.7 us/tile
at 2.4 GHz; 16 tiles/core ~ 284 us PE floor.
"""

import numpy as np

B = 65536
D = 1024
E = 8
H = 128
C1 = 64
NCLS = 2
NCORES = 8
BLOC = B // NCORES  # 8192
NTILE = 512
F = E * H  # 1024
KD = D // 128  # 8 k-chunks over D
MH = F // 128  # 8 h-blocks
NMBLK = MH + 1  # + replicated-gate block

MM_DT = "bfloat16"  # matmul dtype: float32r | bfloat16

import os as _os

# bisect toggles (read at build time)
INTERLEAVE_LAST = _os.environ.get("K_INTERLEAVE_LAST", "1") == "1"
PACKED_X = _os.environ.get("K_PACKED_X", "1") == "1"
WARM_MM = int(_os.environ.get("K_WARM_MM", "36"))

_BUILT = {}


def _np_store_dt(mm_dt_name):
    import ml_dtypes

    return np.float32 if mm_dt_name == "float32r" else ml_dtypes.bfloat16


def _build_nc(b_per_core: int, mm_dt_name: str, repeat: int = 1):
    """Build + compile the Bass module for one core (SPMD across 8).

    repeat > 1 wraps the whole batch loop in a hardware For_i loop that
    re-runs the identical work `repeat` times -- used only for timing
    (amortizes the ~45-90 ms axon dispatch/polling quantum away).
    """
    import concourse.bacc as bacc
    import concourse.tile as tile
    import concourse.mybir as mybir
    from contextlib import nullcontext

    nbt = b_per_core // NTILE
    assert nbt >= 3
    fp32 = mybir.dt.float32
    st_dt = getattr(mybir.dt, mm_dt_name)

    nc = bacc.Bacc("TRN2", target_bir_lowering=False, debug=False)

    # x is packed tile-major on host: xP[p, t, k, c] = x[t*512+c, k*128+p]
    # so one btile = one contiguous-per-partition 8KB DMA.
    xT = nc.dram_tensor(
        "xT", [128, nbt * KD * NTILE], st_dt, kind="ExternalInput"
    )
    w1 = nc.dram_tensor("W1T", [128, NMBLK * KD * 128], st_dt, kind="ExternalInput")
    vb = nc.dram_tensor("Vb", [128, MH * C1], st_dt, kind="ExternalInput")
    s8 = nc.dram_tensor("S8", [8, C1 + 128], st_dt, kind="ExternalInput")
    wc2 = nc.dram_tensor("WC2", [C1, NCLS], st_dt, kind="ExternalInput")
    # per-partition bias columns (fp32): 0..7 = be1 block m, 8 = bg_rep,
    # 9 = bc1 (rows 0:64), 10 = bc2 (rows 0:2)
    bcol = nc.dram_tensor("BCOL", [128, 11], fp32, kind="ExternalInput")
    yT = nc.dram_tensor("yT", [NCLS, b_per_core], fp32, kind="ExternalOutput")

    AF = mybir.ActivationFunctionType
    OP = mybir.AluOpType

    with tile.TileContext(nc) as tc:
        with (
            tc.tile_pool(name="wpool", bufs=1) as wpool,
            tc.tile_pool(name="xpool", bufs=3) as xpool,
            tc.tile_pool(name="spool", bufs=2) as spool,
            tc.tile_pool(name="hpool", bufs=2) as hpool,
            tc.tile_pool(name="opool", bufs=2) as opool,
            tc.tile_pool(name="ps_blk", bufs=4, space="PSUM") as ps_blk,
            tc.tile_pool(name="ps_srep", bufs=1, space="PSUM") as ps_srep,
            tc.tile_pool(name="ps_pre", bufs=2, space="PSUM") as ps_pre,
            tc.tile_pool(name="ps_out", bufs=1, space="PSUM") as ps_out,
        ):
            # ---- PE pre-warm: the HAM clock gate keeps PE at 1.2 GHz until
            # ~3.4us of sustained activity; the first ~10us of the kernel is
            # DMA boot/fill with PE idle, so real matmuls would run the whole
            # first tile cold.  Dependency-free dummy matmuls on a memset
            # tile bridge the gap (results discarded into a scratch bank).
            warm = wpool.tile([128, NTILE], st_dt, tag="warm")
            nc.any.memset(warm[:], 0)
            warm_ps = ps_srep.tile([128, NTILE], fp32, tag="srep", name="warm_ps")
            for _ in range(WARM_MM):
                nc.tensor.matmul(
                    warm_ps[:], warm[:, 0:128], warm[:], start=True, stop=True
                )

            # ---- load weights/constants once (ordered by first use) ----
            w1t = wpool.tile([128, NMBLK * KD * 128], st_dt, tag="w1t")
            bct = wpool.tile([128, 11], fp32, tag="bct")
            s8t = wpool.tile([8, C1 + 128], st_dt, tag="s8t")
            vbt = wpool.tile([128, MH * C1], st_dt, tag="vbt")
            wc2t = wpool.tile([C1, NCLS], st_dt, tag="wc2t")

            def w1dma(m_, half=None):
                c0 = m_ * KD * 128
                n = KD * 128
                if half is not None:
                    c0 += half * (n // 2)
                    n //= 2
                nc.sync.dma_start(w1t[:, c0 : c0 + n], w1[:, c0 : c0 + n])

            TW = KD * NTILE  # 4096 columns per packed btile

            def xdma(t, nparts=1):
                """One packed-btile DMA (or nparts sub-DMAs for startup)."""
                xt_ = xpool.tile([128, TW], st_dt, tag="x")
                step = TW // nparts
                for i in range(nparts):
                    nc.sync.dma_start(
                        xt_[:, i * step : (i + 1) * step],
                        xT[:, t * TW + i * step : t * TW + (i + 1) * step],
                    )
                return xt_

            def w1blk(m, k):
                c0 = (m * KD + k) * 128
                return w1t[:, c0 : c0 + 128]

            c_blk = s8t[:, 0:C1]  # [8, 64]   be2 @ Wc1
            ones8 = s8t[:, C1 : C1 + 128]  # [8, 128] ones

            consts = dict(
                w1blk=w1blk, c_blk=c_blk, ones8=ones8, vbt=vbt, wc2t=wc2t,
                bct=bct, xdma=xdma, w1dma=w1dma, vb=vb, wc2=wc2,
                bctd=(bct, bcol), s8d=(s8t, s8),
                xsrc=lambda t, a, b: xT[:, t * TW + a : t * TW + b],
            )
            if repeat > 1:
                # weights resident across For_i iterations
                w1dma(MH)
                nc.sync.dma_start(bct[:], bcol[:])
                nc.sync.dma_start(s8t[:], s8[:])
                for m_ in range(MH):
                    w1dma(m_)
                nc.sync.dma_start(vbt[:], vb[:])
                nc.sync.dma_start(wc2t[:], wc2[:])
            rep_ctx = tc.For_i(0, repeat, 1) if repeat > 1 else nullcontext()
            with rep_ctx:
                _kernel_body(nc, mybir, nbt, st_dt, xpool, spool, hpool, opool,
                             ps_blk, ps_srep, ps_pre, ps_out, yT, consts,
                             first=(repeat == 1))

    nc.compile()
    return nc


def _kernel_body(nc, mybir, nbt, st_dt, xpool, spool, hpool, opool,
                 ps_blk, ps_srep, ps_pre, ps_out, yT, consts, first):
    AF = mybir.ActivationFunctionType
    OP = mybir.AluOpType
    fp32 = mybir.dt.float32
    w1blk = consts["w1blk"]
    bct = consts["bct"]
    xdma = consts["xdma"]
    w1dma = consts["w1dma"]

    xk = [None] * nbt
    if first:
        # Startup choreography: the gate block needs only the gate weights
        # plus progressive x0 quarters; interleave so nothing queues behind
        # bulk transfers it doesn't need yet, and pull w1_m0 ahead of the
        # late x0 quarters (layer-1 m0 starts ~1.7us after the gate).
        bct_t, bcol_d = consts["bctd"]
        s8t_t, s8_d = consts["s8d"]
        w1dma(MH)
        xt0 = xpool.tile([128, KD * NTILE], st_dt, tag="x")
        Q = KD * NTILE // 4
        nc.sync.dma_start(xt0[:, 0:Q], consts["xsrc"](0, 0, Q))
        nc.sync.dma_start(xt0[:, Q : 2 * Q], consts["xsrc"](0, Q, 2 * Q))
        w1dma(0)
        nc.sync.dma_start(bct_t[:], bcol_d[:])
        nc.sync.dma_start(xt0[:, 2 * Q : 3 * Q], consts["xsrc"](0, 2 * Q, 3 * Q))
        nc.sync.dma_start(xt0[:, 3 * Q : 4 * Q], consts["xsrc"](0, 3 * Q, 4 * Q))
        nc.sync.dma_start(s8t_t[:], s8_d[:])
        xk[0] = xt0
        for m_ in range(1, MH):
            w1dma(m_)
        xk[1] = xdma(1)
        nc.sync.dma_start(consts["vbt"][:], consts["vb"][:])
        nc.sync.dma_start(consts["wc2t"][:], consts["wc2"][:])
    else:
        xk[0] = xdma(0)
        xk[1] = xdma(1)

    def emit_vfold(st):
        """PE: 8 Vb matmuls + c_blk matmul -> ps_pre; ACT relu -> rp."""
        pp = ps_pre.tile([C1, NTILE], fp32, tag="pre")
        for k in range(MH):
            nc.tensor.matmul(
                pp[:], consts["vbt"][:, k * C1 : (k + 1) * C1], st["hs"][k][:],
                start=(k == 0), stop=False,
            )
        nc.tensor.matmul(
            pp[:], consts["c_blk"], st["gw"][0:8, :], start=False, stop=True
        )
        rp = spool.tile([C1, NTILE], st_dt, tag="rp")
        nc.scalar.activation(rp[:], pp[:], AF.Relu, bias=bct[0:C1, 9:10])
        st["rp"] = rp

    def emit_cls2(st):
        op_ = ps_out.tile([NCLS, NTILE], fp32, tag="out")
        nc.tensor.matmul(
            op_[:], consts["wc2t"][:], st["rp"][:], start=True, stop=True
        )
        ot = opool.tile([NCLS, NTILE], fp32, tag="o")
        nc.scalar.activation(ot[:], op_[:], AF.Identity, bias=bct[0:NCLS, 10:11])
        b0 = st["t"] * NTILE
        nc.sync.dma_start(yT[0:NCLS, b0 : b0 + NTILE], ot[:])

    p1 = None  # tile t-1 state (awaiting V-fold)
    p2 = None  # tile t-2 state (awaiting cls2)
    for t in range(nbt):
        if t + 2 < nbt:
            xk[t + 2] = xdma(t + 2)

        # ---- gate block: replicated logits; exp(logit + bg) on ACT ----
        gp = ps_blk.tile([128, NTILE], fp32, tag="blk")
        for k in range(KD):
            nc.tensor.matmul(
                gp[:], w1blk(MH, k), xk[t][:, k * NTILE : (k + 1) * NTILE],
                start=(k == 0), stop=(k == KD - 1),
            )
        expg = spool.tile([128, NTILE], st_dt, tag="expg")
        nc.scalar.activation(expg[:], gp[:], AF.Exp, bias=bct[:, 8:9])

        # ---- layer-1 h-blocks: relu(.+be1) on ACT; gate*h on DVE ----
        # The softmax denom matmul slots between h-blocks 0 and 1: by then
        # the ACT exp is long done (no PE stall), and the DVE chain
        # (1/s -> gw -> 8 gate mults) overlaps h-blocks 1..7, so hs(t) is
        # complete right when the PE reaches V-fold(t) one tile later.
        # For the FINAL tile the V-fold matmuls interleave directly into the
        # layer-1 stream (disjoint PSUM banks, lag 2 blocks behind the hs
        # chain) so the epilogue exposes only the last cls chain.
        last = t == nbt - 1
        pp_last = (
            ps_pre.tile([C1, NTILE], fp32, tag="pre", name="pp_last")
            if last
            else None
        )
        hs = []
        gw = None
        for m in range(MH):
            hp = ps_blk.tile([128, NTILE], fp32, tag="blk")
            for k in range(KD):
                nc.tensor.matmul(
                    hp[:], w1blk(m, k), xk[t][:, k * NTILE : (k + 1) * NTILE],
                    start=(k == 0), stop=(k == KD - 1),
                )
            hr = hpool.tile([128, NTILE], st_dt, tag=f"hs{m}")
            nc.scalar.activation(hr[:], hp[:], AF.Relu, bias=bct[:, m : m + 1])
            hs.append(hr)
            if m == 0:
                sp = ps_srep.tile([128, NTILE], fp32, tag="srep")
                nc.tensor.matmul(
                    sp[:], consts["ones8"], expg[0:8, :], start=True, stop=True
                )
                rinv = spool.tile([128, NTILE], fp32, tag="rinv")
                nc.vector.reciprocal(rinv[:], sp[:])
                gw = spool.tile([128, NTILE], st_dt, tag="gw")
                nc.vector.tensor_tensor(gw[:], expg[:], rinv[:], op=OP.mult)
            nc.vector.tensor_tensor(hs[m][:], hs[m][:], gw[:], op=OP.mult)
            if last and m >= 2:
                nc.tensor.matmul(
                    pp_last[:],
                    consts["vbt"][:, (m - 2) * C1 : (m - 1) * C1],
                    hs[m - 2][:],
                    start=(m == 2), stop=False, skip_group_check=True,
                )
        xk[t] = None

        # ---- pipelined tails: V-fold(t-1) then cls2(t-2) on PE ----
        if p1 is not None:
            emit_vfold(p1)
        if p2 is not None:
            emit_cls2(p2)
        p2 = p1
        p1 = {"t": t, "hs": hs, "gw": gw}

    # epilogue: finish the last tile's interleaved V-fold, then drain the
    # cls chain in column halves so ACT/PE/DMA pipeline instead of
    # serializing on the full-width pre -> rp -> cls2 -> out chain.
    HN = NTILE // 2
    for k in range(MH - 2, MH):
        nc.tensor.matmul(
            pp_last[:], consts["vbt"][:, k * C1 : (k + 1) * C1], hs[k][:],
            start=False, stop=False, skip_group_check=True,
        )
    for h_ in range(2):
        nc.tensor.matmul(
            pp_last[:, h_ * HN : (h_ + 1) * HN], consts["c_blk"],
            gw[0:8, h_ * HN : (h_ + 1) * HN],
            start=False, stop=True, skip_group_check=True,
        )
    emit_cls2(p2)  # fills the PE gap while rp half 0 is on ACT
    rp = spool.tile([C1, NTILE], st_dt, tag="rp")
    op_ = ps_out.tile([NCLS, NTILE], fp32, tag="out", name="op_last")
    ot = opool.tile([NCLS, NTILE], fp32, tag="o", name="ot_last")
    b0 = p1["t"] * NTILE
    for h_ in range(2):
        sl = slice(h_ * HN, (h_ + 1) * HN)
        nc.scalar.activation(
            rp[:, sl], pp_last[:, sl], AF.Relu, bias=bct[0:C1, 9:10]
        )
        nc.tensor.matmul(
            op_[:, sl], consts["wc2t"][:], rp[:, sl],
            start=True, stop=True, skip_group_check=True,
        )
        nc.scalar.activation(
            ot[:, sl], op_[:, sl], AF.Identity, bias=bct[0:NCLS, 10:11]
        )
        nc.sync.dma_start(
            yT[0:NCLS, b0 + h_ * HN : b0 + (h_ + 1) * HN], ot[:, sl]
        )


def _get_nc(b_per_core: int, mm_dt_name: str, repeat: int = 1):
    key = (b_per_core, mm_dt_name, repeat)
    if key not in _BUILT:
        _BUILT[key] = _build_nc(b_per_core, mm_dt_name, repeat)
    return _BUILT[key]


def prep_inputs(x, We1, be1, We2, be2, Wg, bg, Wc1, bc1, Wc2, bc2,
                mm_dt_name=MM_DT, n_cores=NCORES):
    """Host-side packing -> list of per-core input maps."""
    f64 = np.float64
    sdt = _np_store_dt(mm_dt_name)
    b_per_core = x.shape[0] // n_cores

    # feature order f = j*E + e
    W1_all = np.transpose(np.asarray(We1, f64), (1, 2, 0)).reshape(D, F)
    Wg_rep = np.asarray(Wg, f64)[:, np.arange(128) % E]
    blocks = []
    for m_ in range(MH):
        for k in range(KD):
            blocks.append(W1_all[k * 128 : (k + 1) * 128, m_ * 128 : (m_ + 1) * 128])
    for k in range(KD):
        blocks.append(Wg_rep[k * 128 : (k + 1) * 128, :])
    W1T = np.ascontiguousarray(np.concatenate(blocks, axis=1).astype(sdt))

    V = np.einsum("ejk,kc->jec", np.asarray(We2, f64), np.asarray(Wc1, f64)).reshape(
        F, C1
    )
    Vb = np.ascontiguousarray(
        np.concatenate([V[k * 128 : (k + 1) * 128, :] for k in range(MH)], axis=1)
        .astype(sdt)
    )
    Cm = np.asarray(be2, f64) @ np.asarray(Wc1, f64)  # [E, C1]
    S8 = np.ascontiguousarray(
        np.concatenate([Cm, np.ones((E, 128), f64)], axis=1).astype(sdt)
    )
    WC2 = np.ascontiguousarray(np.asarray(Wc2, f64).astype(sdt))

    bcol = np.zeros((128, 11), np.float32)
    be1_int = np.asarray(be1, f64).T.reshape(F)  # f = j*E + e
    for m_ in range(MH):
        bcol[:, m_] = be1_int[m_ * 128 : (m_ + 1) * 128]
    bcol[:, 8] = np.asarray(bg, f64)[np.arange(128) % E]
    bcol[0:C1, 9] = np.asarray(bc1, f64)
    bcol[0:NCLS, 10] = np.asarray(bc2, f64)

    # pack x tile-major: xP[p, t, k, c] = x[t*NTILE+c, k*128+p] per core, so
    # each 512-row btile is one contiguous-per-partition 8KB DMA.
    nbt = b_per_core // NTILE
    xT_full = np.asarray(x).T.astype(sdt)  # [D, B]
    in_maps = []
    for c in range(n_cores):
        xc = xT_full[:, c * b_per_core : (c + 1) * b_per_core]
        xP = np.ascontiguousarray(
            xc.reshape(KD, 128, nbt, NTILE)
            .transpose(1, 2, 0, 3)
            .reshape(128, nbt * KD * NTILE)
        )
        in_maps.append(
            {
                "xT": xP,
                "W1T": W1T,
                "Vb": Vb,
                "S8": S8,
                "WC2": WC2,
                "BCOL": bcol,
            }
        )
    return in_maps, b_per_core


def run(inputs, mm_dt_name=MM_DT, trace=False):
    """Run on 8 NeuronCores; returns (y [B, 2] fp32, exec_time_ns or None)."""
    from concourse.bass_utils import run_bass_kernel_spmd

    in_maps, b_per_core = prep_inputs(**inputs, mm_dt_name=mm_dt_name)
    nc = _get_nc(b_per_core, mm_dt_name)
    res = run_bass_kernel_spmd(
        nc, in_maps, core_ids=list(range(NCORES)), trace=trace
    )
    y = np.concatenate([r["yT"].T for r in res.results], axis=0)
    return np.ascontiguousarray(y.astype(np.float32)), res.exec_time_ns


def kernel(**inputs):
    y, _ = run(inputs)
    return y


# revision 25
# speedup vs baseline: 1.0172x; 1.0172x over previous
"""Trainium2 Bass kernel for nn_HVGuardModel (dense MoE routing).

Reference math (B=65536, D=1024, E=8, H=128, C1=64, NC=2):
    gw  = softmax(x @ Wg + bg)                      [B, E]
    h   = relu(einsum('bd,edh', x, We1) + be1)      [B, E, H]
    eo  = einsum('beh,eho', h, We2) + be2           [B, E, H]
    mix = einsum('be,beh', gw, eo)                  [B, H]
    out = relu(mix @ Wc1 + bc1) @ Wc2 + bc2         [B, NC]

Strategy: pure data-parallel over 8 cores (8192 rows each).  All device
activations live in "feature-major" layout [feature, batch]; the host
supplies x pre-transposed (xT) and transposes the [2, 8192] per-core
outputs back.

Algebraic folds (host side):
  * mix is only consumed via mix @ Wc1  =>  fold V = We2 @ Wc1 per expert
    ([E*H, 64] stacked) and C = be2 @ Wc1; eo and mix are never materialized.
  * Layer-1 features are INTERLEAVED: f = j*E + e.  A "replicated gate"
    weight block (Wg columns tiled mod 8) yields a [128, N] logit tile whose
    row r holds logit[r mod 8], so no cross-partition broadcast is needed.
  * softmax denominator: all-ones [8,128] lhsT matmul replicates
    s = sum_e exp across all 128 partitions; 1/s on DVE.
  * All biases are per-partition -> ACT's native bias operand.

v2 over the fp32r baseline (373 us):
  * bf16 matmul operands everywhere.  Cost-model cycles/row are identical,
    but on HW fp32r matmuls must self-load their stationary operand (bass
    refuses standalone fp32r ldweights), serializing the weight load with
    the stream; bf16 gets a separate LDWEIGHTS the PE pulls ahead.  Also
    halves x DMA traffic (33.5 -> 16.8 MB/core) and doubles DVE throughput.
  * Software-pipelined PE program:  per tile t the PE order is
        gate(t), layer1(t), denom(t), Vfold(t-1), cls2(t-2)
    so every matmul's ACT/DVE-produced operands (exp -> 1/s -> gw -> hs)
    are computed a full tile (~18 us) before the PE consumes them; the
    softmax chain never stalls the PE.
  * x tiles prefetched 2 deep (xpool bufs=3).

Per 512-column batch tile: 83 matmuls = 42,496 PE columns -> 17.7 us/tile
at 2.4 GHz; 16 tiles/core ~ 284 us PE floor.
"""

import numpy as np

B = 65536
D = 1024
E = 8
H = 128
C1 = 64
NCLS = 2
NCORES = 8
BLOC = B // NCORES  # 8192
NTILE = 512
F = E * H  # 1024
KD = D // 128  # 8 k-chunks over D
MH = F // 128  # 8 h-blocks
NMBLK = MH + 1  # + replicated-gate block

MM_DT = "bfloat16"  # matmul dtype: float32r | bfloat16

import os as _os

# bisect toggles (read at build time)
INTERLEAVE_LAST = _os.environ.get("K_INTERLEAVE_LAST", "1") == "1"
PACKED_X = _os.environ.get("K_PACKED_X", "1") == "1"
WARM_MM = int(_os.environ.get("K_WARM_MM", "10"))

_BUILT = {}


def _np_store_dt(mm_dt_name):
    import ml_dtypes

    return np.float32 if mm_dt_name == "float32r" else ml_dtypes.bfloat16


def _build_nc(b_per_core: int, mm_dt_name: str, repeat: int = 1):
    """Build + compile the Bass module for one core (SPMD across 8).

    repeat > 1 wraps the whole batch loop in a hardware For_i loop that
    re-runs the identical work `repeat` times -- used only for timing
    (amortizes the ~45-90 ms axon dispatch/polling quantum away).
    """
    import concourse.bacc as bacc
    import concourse.tile as tile
    import concourse.mybir as mybir
    from contextlib import nullcontext

    nbt = b_per_core // NTILE
    assert nbt >= 3
    fp32 = mybir.dt.float32
    st_dt = getattr(mybir.dt, mm_dt_name)

    nc = bacc.Bacc("TRN2", target_bir_lowering=False, debug=False)

    # x is packed tile-major on host: xP[p, t, k, c] = x[t*512+c, k*128+p]
    # so one btile = one contiguous-per-partition 8KB DMA.
    xT = nc.dram_tensor(
        "xT", [128, nbt * KD * NTILE], st_dt, kind="ExternalInput"
    )
    w1 = nc.dram_tensor("W1T", [128, NMBLK * KD * 128], st_dt, kind="ExternalInput")
    vb = nc.dram_tensor("Vb", [128, MH * C1], st_dt, kind="ExternalInput")
    s8 = nc.dram_tensor("S8", [8, C1 + 128], st_dt, kind="ExternalInput")
    wc2 = nc.dram_tensor("WC2", [C1, NCLS], st_dt, kind="ExternalInput")
    # per-partition bias columns (fp32): 0..7 = be1 block m, 8 = bg_rep,
    # 9 = bc1 (rows 0:64), 10 = bc2 (rows 0:2)
    bcol = nc.dram_tensor("BCOL", [128, 11], fp32, kind="ExternalInput")
    yT = nc.dram_tensor("yT", [NCLS, b_per_core], fp32, kind="ExternalOutput")

    AF = mybir.ActivationFunctionType
    OP = mybir.AluOpType

    with tile.TileContext(nc) as tc:
        with (
            tc.tile_pool(name="wpool", bufs=1) as wpool,
            tc.tile_pool(name="xpool", bufs=3) as xpool,
            tc.tile_pool(name="spool", bufs=2) as spool,
            tc.tile_pool(name="hpool", bufs=2) as hpool,
            tc.tile_pool(name="opool", bufs=2) as opool,
            tc.tile_pool(name="ps_blk", bufs=4, space="PSUM") as ps_blk,
            tc.tile_pool(name="ps_srep", bufs=1, space="PSUM") as ps_srep,
            tc.tile_pool(name="ps_pre", bufs=2, space="PSUM") as ps_pre,
            tc.tile_pool(name="ps_out", bufs=1, space="PSUM") as ps_out,
        ):
            # ---- PE pre-warm: the HAM clock gate keeps PE at 1.2 GHz until
            # ~3.4us of sustained activity; the first ~10us of the kernel is
            # DMA boot/fill with PE idle, so real matmuls would run the whole
            # first tile cold.  Dependency-free dummy matmuls on a memset
            # tile bridge the gap (results discarded into a scratch bank).
            warm = wpool.tile([128, NTILE], st_dt, tag="warm")
            nc.any.memset(warm[:], 0)
            warm_ps = ps_srep.tile([128, NTILE], fp32, tag="srep", name="warm_ps")
            for _ in range(WARM_MM):
                nc.tensor.matmul(
                    warm_ps[:], warm[:, 0:128], warm[:], start=True, stop=True
                )

            # ---- load weights/constants once (ordered by first use) ----
            w1t = wpool.tile([128, NMBLK * KD * 128], st_dt, tag="w1t")
            bct = wpool.tile([128, 11], fp32, tag="bct")
            s8t = wpool.tile([8, C1 + 128], st_dt, tag="s8t")
            vbt = wpool.tile([128, MH * C1], st_dt, tag="vbt")
            wc2t = wpool.tile([C1, NCLS], st_dt, tag="wc2t")

            def w1dma(m_, half=None):
                c0 = m_ * KD * 128
                n = KD * 128
                if half is not None:
                    c0 += half * (n // 2)
                    n //= 2
                nc.sync.dma_start(w1t[:, c0 : c0 + n], w1[:, c0 : c0 + n])

            TW = KD * NTILE  # 4096 columns per packed btile

            def xdma(t, nparts=1):
                """One packed-btile DMA (or nparts sub-DMAs for startup)."""
                xt_ = xpool.tile([128, TW], st_dt, tag="x")
                step = TW // nparts
                for i in range(nparts):
                    nc.sync.dma_start(
                        xt_[:, i * step : (i + 1) * step],
                        xT[:, t * TW + i * step : t * TW + (i + 1) * step],
                    )
                return xt_

            def w1blk(m, k):
                c0 = (m * KD + k) * 128
                return w1t[:, c0 : c0 + 128]

            c_blk = s8t[:, 0:C1]  # [8, 64]   be2 @ Wc1
            ones8 = s8t[:, C1 : C1 + 128]  # [8, 128] ones

            consts = dict(
                w1blk=w1blk, c_blk=c_blk, ones8=ones8, vbt=vbt, wc2t=wc2t,
                bct=bct, xdma=xdma, w1dma=w1dma, vb=vb, wc2=wc2,
                bctd=(bct, bcol), s8d=(s8t, s8),
                xsrc=lambda t, a, b: xT[:, t * TW + a : t * TW + b],
            )
            if repeat > 1:
                # weights resident across For_i iterations
                w1dma(MH)
                nc.sync.dma_start(bct[:], bcol[:])
                nc.sync.dma_start(s8t[:], s8[:])
                for m_ in range(MH):
                    w1dma(m_)
                nc.sync.dma_start(vbt[:], vb[:])
                nc.sync.dma_start(wc2t[:], wc2[:])
            rep_ctx = tc.For_i(0, repeat, 1) if repeat > 1 else nullcontext()
            with rep_ctx:
                _kernel_body(nc, mybir, nbt, st_dt, xpool, spool, hpool, opool,
                             ps_blk, ps_srep, ps_pre, ps_out, yT, consts,
                             first=(repeat == 1))

    nc.compile()
    return nc


def _kernel_body(nc, mybir, nbt, st_dt, xpool, spool, hpool, opool,
                 ps_blk, ps_srep, ps_pre, ps_out, yT, consts, first):
    AF = mybir.ActivationFunctionType
    OP = mybir.AluOpType
    fp32 = mybir.dt.float32
    w1blk = consts["w1blk"]
    bct = consts["bct"]
    xdma = consts["xdma"]
    w1dma = consts["w1dma"]

    xk = [None] * nbt
    if first:
        # Startup choreography: the gate block needs only the gate weights
        # plus progressive x0 quarters; interleave so nothing queues behind
        # bulk transfers it doesn't need yet, and pull w1_m0 ahead of the
        # late x0 quarters (layer-1 m0 starts ~1.7us after the gate).
        bct_t, bcol_d = consts["bctd"]
        s8t_t, s8_d = consts["s8d"]
        w1dma(MH)
        xt0 = xpool.tile([128, KD * NTILE], st_dt, tag="x")
        Q = KD * NTILE // 4
        nc.sync.dma_start(xt0[:, 0:Q], consts["xsrc"](0, 0, Q))
        nc.sync.dma_start(xt0[:, Q : 2 * Q], consts["xsrc"](0, Q, 2 * Q))
        w1dma(0)
        nc.sync.dma_start(bct_t[:], bcol_d[:])
        nc.sync.dma_start(xt0[:, 2 * Q : 3 * Q], consts["xsrc"](0, 2 * Q, 3 * Q))
        nc.sync.dma_start(xt0[:, 3 * Q : 4 * Q], consts["xsrc"](0, 3 * Q, 4 * Q))
        nc.sync.dma_start(s8t_t[:], s8_d[:])
        xk[0] = xt0
        for m_ in range(1, MH):
            w1dma(m_)
        xk[1] = xdma(1)
        nc.sync.dma_start(consts["vbt"][:], consts["vb"][:])
        nc.sync.dma_start(consts["wc2t"][:], consts["wc2"][:])
    else:
        xk[0] = xdma(0)
        xk[1] = xdma(1)

    def emit_vfold(st):
        """PE: 8 Vb matmuls + c_blk matmul -> ps_pre; ACT relu -> rp."""
        pp = ps_pre.tile([C1, NTILE], fp32, tag="pre")
        for k in range(MH):
            nc.tensor.matmul(
                pp[:], consts["vbt"][:, k * C1 : (k + 1) * C1], st["hs"][k][:],
                start=(k == 0), stop=False,
            )
        nc.tensor.matmul(
            pp[:], consts["c_blk"], st["gw"][0:8, :], start=False, stop=True
        )
        rp = spool.tile([C1, NTILE], st_dt, tag="rp")
        nc.scalar.activation(rp[:], pp[:], AF.Relu, bias=bct[0:C1, 9:10])
        st["rp"] = rp

    def emit_cls2(st):
        op_ = ps_out.tile([NCLS, NTILE], fp32, tag="out")
        nc.tensor.matmul(
            op_[:], consts["wc2t"][:], st["rp"][:], start=True, stop=True
        )
        ot = opool.tile([NCLS, NTILE], fp32, tag="o")
        nc.scalar.activation(ot[:], op_[:], AF.Identity, bias=bct[0:NCLS, 10:11])
        b0 = st["t"] * NTILE
        nc.sync.dma_start(yT[0:NCLS, b0 : b0 + NTILE], ot[:])

    p1 = None  # tile t-1 state (awaiting V-fold)
    p2 = None  # tile t-2 state (awaiting cls2)
    for t in range(nbt):
        if t + 2 < nbt:
            xk[t + 2] = xdma(t + 2)

        # ---- gate block: replicated logits; exp(logit + bg) on ACT ----
        gp = ps_blk.tile([128, NTILE], fp32, tag="blk")
        for k in range(KD):
            nc.tensor.matmul(
                gp[:], w1blk(MH, k), xk[t][:, k * NTILE : (k + 1) * NTILE],
                start=(k == 0), stop=(k == KD - 1),
            )
        expg = spool.tile([128, NTILE], st_dt, tag="expg")
        nc.scalar.activation(expg[:], gp[:], AF.Exp, bias=bct[:, 8:9])

        # ---- layer-1 h-blocks: relu(.+be1) on ACT; gate*h on DVE ----
        # The softmax denom matmul slots between h-blocks 0 and 1: by then
        # the ACT exp is long done (no PE stall), and the DVE chain
        # (1/s -> gw -> 8 gate mults) overlaps h-blocks 1..7, so hs(t) is
        # complete right when the PE reaches V-fold(t) one tile later.
        # For the FINAL tile the V-fold matmuls interleave directly into the
        # layer-1 stream (disjoint PSUM banks, lag 2 blocks behind the hs
        # chain) so the epilogue exposes only the last cls chain.
        last = t == nbt - 1
        pp_last = (
            ps_pre.tile([C1, NTILE], fp32, tag="pre", name="pp_last")
            if last
            else None
        )
        hs = []
        gw = None
        for m in range(MH):
            hp = ps_blk.tile([128, NTILE], fp32, tag="blk")
            for k in range(KD):
                nc.tensor.matmul(
                    hp[:], w1blk(m, k), xk[t][:, k * NTILE : (k + 1) * NTILE],
                    start=(k == 0), stop=(k == KD - 1),
                )
            hr = hpool.tile([128, NTILE], st_dt, tag=f"hs{m}")
            nc.scalar.activation(hr[:], hp[:], AF.Relu, bias=bct[:, m : m + 1])
            hs.append(hr)
            if m == 0:
                sp = ps_srep.tile([128, NTILE], fp32, tag="srep")
                nc.tensor.matmul(
                    sp[:], consts["ones8"], expg[0:8, :], start=True, stop=True
                )
                rinv = spool.tile([128, NTILE], fp32, tag="rinv")
                nc.vector.reciprocal(rinv[:], sp[:])
                gw = spool.tile([128, NTILE], st_dt, tag="gw")
                nc.vector.tensor_tensor(gw[:], expg[:], rinv[:], op=OP.mult)
            nc.vector.tensor_tensor(hs[m][:], hs[m][:], gw[:], op=OP.mult)
            if last and m >= 2:
                nc.tensor.matmul(
                    pp_last[:],
                    consts["vbt"][:, (m - 2) * C1 : (m - 1) * C1],
                    hs[m - 2][:],
                    start=(m == 2), stop=False, skip_group_check=True,
                )
        xk[t] = None

        # ---- pipelined tails: V-fold(t-1) then cls2(t-2) on PE ----
        if p1 is not None:
            emit_vfold(p1)
        if p2 is not None:
            emit_cls2(p2)
        p2 = p1
        p1 = {"t": t, "hs": hs, "gw": gw}

    # epilogue: finish the last tile's interleaved V-fold, then drain the
    # cls chain in column halves so ACT/PE/DMA pipeline instead of
    # serializing on the full-width pre -> rp -> cls2 -> out chain.
    HN = NTILE // 2
    for k in range(MH - 2, MH):
        nc.tensor.matmul(
            pp_last[:], consts["vbt"][:, k * C1 : (k + 1) * C1], hs[k][:],
            start=False, stop=False, skip_group_check=True,
        )
    for h_ in range(2):
        nc.tensor.matmul(
            pp_last[:, h_ * HN : (h_ + 1) * HN], consts["c_blk"],
            gw[0:8, h_ * HN : (h_ + 1) * HN],
            start=False, stop=True, skip_group_check=True,
        )
    emit_cls2(p2)  # fills the PE gap while rp half 0 is on ACT
    rp = spool.tile([C1, NTILE], st_dt, tag="rp")
    op_ = ps_out.tile([NCLS, NTILE], fp32, tag="out", name="op_last")
    ot = opool.tile([NCLS, NTILE], fp32, tag="o", name="ot_last")
    b0 = p1["t"] * NTILE
    for h_ in range(2):
        sl = slice(h_ * HN, (h_ + 1) * HN)
        nc.scalar.activation(
            rp[:, sl], pp_last[:, sl], AF.Relu, bias=bct[0:C1, 9:10]
        )
        nc.tensor.matmul(
            op_[:, sl], consts["wc2t"][:], rp[:, sl],
            start=True, stop=True, skip_group_check=True,
        )
        nc.scalar.activation(
            ot[:, sl], op_[:, sl], AF.Identity, bias=bct[0:NCLS, 10:11]
        )
        nc.sync.dma_start(
            yT[0:NCLS, b0 + h_ * HN : b0 + (h_ + 1) * HN], ot[:, sl]
        )


def _get_nc(b_per_core: int, mm_dt_name: str, repeat: int = 1):
    key = (b_per_core, mm_dt_name, repeat)
    if key not in _BUILT:
        _BUILT[key] = _build_nc(b_per_core, mm_dt_name, repeat)
    return _BUILT[key]


def prep_inputs(x, We1, be1, We2, be2, Wg, bg, Wc1, bc1, Wc2, bc2,
                mm_dt_name=MM_DT, n_cores=NCORES):
    """Host-side packing -> list of per-core input maps."""
    f64 = np.float64
    sdt = _np_store_dt(mm_dt_name)
    b_per_core = x.shape[0] // n_cores

    # feature order f = j*E + e
    W1_all = np.transpose(np.asarray(We1, f64), (1, 2, 0)).reshape(D, F)
    Wg_rep = np.asarray(Wg, f64)[:, np.arange(128) % E]
    blocks = []
    for m_ in range(MH):
        for k in range(KD):
            blocks.append(W1_all[k * 128 : (k + 1) * 128, m_ * 128 : (m_ + 1) * 128])
    for k in range(KD):
        blocks.append(Wg_rep[k * 128 : (k + 1) * 128, :])
    W1T = np.ascontiguousarray(np.concatenate(blocks, axis=1).astype(sdt))

    V = np.einsum("ejk,kc->jec", np.asarray(We2, f64), np.asarray(Wc1, f64)).reshape(
        F, C1
    )
    Vb = np.ascontiguousarray(
        np.concatenate([V[k * 128 : (k + 1) * 128, :] for k in range(MH)], axis=1)
        .astype(sdt)
    )
    Cm = np.asarray(be2, f64) @ np.asarray(Wc1, f64)  # [E, C1]
    S8 = np.ascontiguousarray(
        np.concatenate([Cm, np.ones((E, 128), f64)], axis=1).astype(sdt)
    )
    WC2 = np.ascontiguousarray(np.asarray(Wc2, f64).astype(sdt))

    bcol = np.zeros((128, 11), np.float32)
    be1_int = np.asarray(be1, f64).T.reshape(F)  # f = j*E + e
    for m_ in range(MH):
        bcol[:, m_] = be1_int[m_ * 128 : (m_ + 1) * 128]
    bcol[:, 8] = np.asarray(bg, f64)[np.arange(128) % E]
    bcol[0:C1, 9] = np.asarray(bc1, f64)
    bcol[0:NCLS, 10] = np.asarray(bc2, f64)

    # pack x tile-major: xP[p, t, k, c] = x[t*NTILE+c, k*128+p] per core, so
    # each 512-row btile is one contiguous-per-partition 8KB DMA.
    nbt = b_per_core // NTILE
    xT_full = np.asarray(x).T.astype(sdt)  # [D, B]
    in_maps = []
    for c in range(n_cores):
        xc = xT_full[:, c * b_per_core : (c + 1) * b_per_core]
        xP = np.ascontiguousarray(
            xc.reshape(KD, 128, nbt, NTILE)
            .transpose(1, 2, 0, 3)
            .reshape(128, nbt * KD * NTILE)
        )
        in_maps.append(
            {
                "xT": xP,
                "W1T": W1T,
                "Vb": Vb,
                "S8": S8,
                "WC2": WC2,
                "BCOL": bcol,
            }
        )
    return in_maps, b_per_core


def run(inputs, mm_dt_name=MM_DT, trace=False):
    """Run on 8 NeuronCores; returns (y [B, 2] fp32, exec_time_ns or None)."""
    from concourse.bass_utils import run_bass_kernel_spmd

    in_maps, b_per_core = prep_inputs(**inputs, mm_dt_name=mm_dt_name)
    nc = _get_nc(b_per_core, mm_dt_name)
    res = run_bass_kernel_spmd(
        nc, in_maps, core_ids=list(range(NCORES)), trace=trace
    )
    y = np.concatenate([r["yT"].T for r in res.results], axis=0)
    return np.ascontiguousarray(y.astype(np.float32)), res.exec_time_ns


def kernel(**inputs):
    y, _ = run(inputs)
    return y


# revision 35
# speedup vs baseline: 1.0470x; 1.0293x over previous
"""Trainium2 Bass kernel for nn_HVGuardModel (dense MoE routing).

Reference math (B=65536, D=1024, E=8, H=128, C1=64, NC=2):
    gw  = softmax(x @ Wg + bg)                      [B, E]
    h   = relu(einsum('bd,edh', x, We1) + be1)      [B, E, H]
    eo  = einsum('beh,eho', h, We2) + be2           [B, E, H]
    mix = einsum('be,beh', gw, eo)                  [B, H]
    out = relu(mix @ Wc1 + bc1) @ Wc2 + bc2         [B, NC]

Strategy: pure data-parallel over 8 cores (8192 rows each).  All device
activations live in "feature-major" layout [feature, batch]; the host
supplies x pre-transposed (xT) and transposes the [2, 8192] per-core
outputs back.

Algebraic folds (host side):
  * mix is only consumed via mix @ Wc1  =>  fold V = We2 @ Wc1 per expert
    ([E*H, 64] stacked) and C = be2 @ Wc1; eo and mix are never materialized.
  * Layer-1 features are INTERLEAVED: f = j*E + e.  A "replicated gate"
    weight block (Wg columns tiled mod 8) yields a [128, N] logit tile whose
    row r holds logit[r mod 8], so no cross-partition broadcast is needed.
  * softmax denominator: all-ones [8,128] lhsT matmul replicates
    s = sum_e exp across all 128 partitions; 1/s on DVE.
  * All biases are per-partition -> ACT's native bias operand.

v2 over the fp32r baseline (373 us):
  * bf16 matmul operands everywhere.  Cost-model cycles/row are identical,
    but on HW fp32r matmuls must self-load their stationary operand (bass
    refuses standalone fp32r ldweights), serializing the weight load with
    the stream; bf16 gets a separate LDWEIGHTS the PE pulls ahead.  Also
    halves x DMA traffic (33.5 -> 16.8 MB/core) and doubles DVE throughput.
  * Software-pipelined PE program:  per tile t the PE order is
        gate(t), layer1(t), denom(t), Vfold(t-1), cls2(t-2)
    so every matmul's ACT/DVE-produced operands (exp -> 1/s -> gw -> hs)
    are computed a full tile (~18 us) before the PE consumes them; the
    softmax chain never stalls the PE.
  * x tiles prefetched 2 deep (xpool bufs=3).

Per 512-column batch tile: 83 matmuls = 42,496 PE columns -> 17.7 us/tile
at 2.4 GHz; 16 tiles/core ~ 284 us PE floor.
"""

import numpy as np

B = 65536
D = 1024
E = 8
H = 128
C1 = 64
NCLS = 2
NCORES = 8
BLOC = B // NCORES  # 8192
NTILE = 512
F = E * H  # 1024
KD = D // 128  # 8 k-chunks over D
MH = F // 128  # 8 h-blocks
NMBLK = MH + 1  # + replicated-gate block

MM_DT = "bfloat16"  # matmul dtype: float32r | bfloat16

import os as _os

# bisect toggles (read at build time)
INTERLEAVE_LAST = _os.environ.get("K_INTERLEAVE_LAST", "1") == "1"
PACKED_X = _os.environ.get("K_PACKED_X", "1") == "1"
WARM_MM = int(_os.environ.get("K_WARM_MM", "10"))

_BUILT = {}


def _np_store_dt(mm_dt_name):
    import ml_dtypes

    return np.float32 if mm_dt_name == "float32r" else ml_dtypes.bfloat16


def _build_nc(b_per_core: int, mm_dt_name: str, repeat: int = 1):
    """Build + compile the Bass module for one core (SPMD across 8).

    repeat > 1 wraps the whole batch loop in a hardware For_i loop that
    re-runs the identical work `repeat` times -- used only for timing
    (amortizes the ~45-90 ms axon dispatch/polling quantum away).
    """
    import concourse.bacc as bacc
    import concourse.tile as tile
    import concourse.mybir as mybir
    from contextlib import nullcontext

    nbt = b_per_core // NTILE
    assert nbt >= 3
    fp32 = mybir.dt.float32
    st_dt = getattr(mybir.dt, mm_dt_name)

    nc = bacc.Bacc("TRN2", target_bir_lowering=False, debug=False)

    # x is packed tile-major on host: xP[p, t, k, c] = x[t*512+c, k*128+p]
    # so one btile = one contiguous-per-partition 8KB DMA.
    xT = nc.dram_tensor(
        "xT", [128, nbt * KD * NTILE], st_dt, kind="ExternalInput"
    )
    # All matmuls are padded to the uniform K=128, M=128 shape: any change
    # of the PE array row/col-group configuration between accumulation
    # groups blocks the background-buffer LDWEIGHTS pull-ahead and costs
    # ~100 ns per transition (~6 transitions/tile measured on HW).  Padding
    # is free: matmul time scales with N only.
    #   Vb = [V blocks padded to [128,128] | tile(Cm)/16 padded]
    #   WC2 padded to [128,128]; softmax denom uses a memset 1/16 constant
    #   against the fully-replicated expg (identical math, K=128).
    w1 = nc.dram_tensor("W1T", [128, NMBLK * KD * 128], st_dt, kind="ExternalInput")
    vb = nc.dram_tensor("Vb", [128, (MH + 1) * 128], st_dt, kind="ExternalInput")
    wc2 = nc.dram_tensor("WC2", [128, 128], st_dt, kind="ExternalInput")
    # per-partition bias columns (fp32): 0..7 = be1 block m, 8 = bg_rep,
    # 9 = bc1 (rows 0:64), 10 = bc2 (rows 0:2)
    bcol = nc.dram_tensor("BCOL", [128, 11], fp32, kind="ExternalInput")
    yT = nc.dram_tensor("yT", [NCLS, b_per_core], fp32, kind="ExternalOutput")

    AF = mybir.ActivationFunctionType
    OP = mybir.AluOpType

    with tile.TileContext(nc) as tc:
        with (
            tc.tile_pool(name="wpool", bufs=1) as wpool,
            tc.tile_pool(name="xpool", bufs=3) as xpool,
            tc.tile_pool(name="spool", bufs=2) as spool,
            tc.tile_pool(name="hpool", bufs=2) as hpool,
            tc.tile_pool(name="opool", bufs=2) as opool,
            tc.tile_pool(name="ps_blk", bufs=4, space="PSUM") as ps_blk,
            tc.tile_pool(name="ps_srep", bufs=1, space="PSUM") as ps_srep,
            tc.tile_pool(name="ps_pre", bufs=2, space="PSUM") as ps_pre,
            tc.tile_pool(name="ps_out", bufs=1, space="PSUM") as ps_out,
        ):
            # ---- PE pre-warm: the HAM clock gate keeps PE at 1.2 GHz until
            # ~3.4us of sustained activity; the first ~10us of the kernel is
            # DMA boot/fill with PE idle, so real matmuls would run the whole
            # first tile cold.  Dependency-free dummy matmuls on a memset
            # tile bridge the gap (results discarded into a scratch bank).
            warm = wpool.tile([128, NTILE], st_dt, tag="warm")
            nc.any.memset(warm[:], 0)
            warm_ps = ps_srep.tile([128, NTILE], fp32, tag="srep", name="warm_ps")
            for _ in range(WARM_MM):
                nc.tensor.matmul(
                    warm_ps[:], warm[:, 0:128], warm[:], start=True, stop=True
                )

            # ---- load weights/constants once (ordered by first use) ----
            w1t = wpool.tile([128, NMBLK * KD * 128], st_dt, tag="w1t")
            bct = wpool.tile([128, 11], fp32, tag="bct")
            ones16 = wpool.tile([128, 128], st_dt, tag="ones16")
            nc.any.memset(ones16[:], 1.0 / 16.0)
            vbt = wpool.tile([128, (MH + 1) * 128], st_dt, tag="vbt")
            wc2t = wpool.tile([128, 128], st_dt, tag="wc2t")

            def w1dma(m_, half=None):
                c0 = m_ * KD * 128
                n = KD * 128
                if half is not None:
                    c0 += half * (n // 2)
                    n //= 2
                nc.sync.dma_start(w1t[:, c0 : c0 + n], w1[:, c0 : c0 + n])

            TW = KD * NTILE  # 4096 columns per packed btile

            def xdma(t, nparts=1):
                """One packed-btile DMA (or nparts sub-DMAs for startup)."""
                xt_ = xpool.tile([128, TW], st_dt, tag="x")
                step = TW // nparts
                for i in range(nparts):
                    nc.sync.dma_start(
                        xt_[:, i * step : (i + 1) * step],
                        xT[:, t * TW + i * step : t * TW + (i + 1) * step],
                    )
                return xt_

            def w1blk(m, k):
                c0 = (m * KD + k) * 128
                return w1t[:, c0 : c0 + 128]

            consts = dict(
                w1blk=w1blk, ones16=ones16, vbt=vbt, wc2t=wc2t,
                bct=bct, xdma=xdma, w1dma=w1dma, vb=vb, wc2=wc2,
                bctd=(bct, bcol),
                xsrc=lambda t, a, b: xT[:, t * TW + a : t * TW + b],
            )
            if repeat > 1:
                # weights resident across For_i iterations
                w1dma(MH)
                nc.sync.dma_start(bct[:], bcol[:])
                for m_ in range(MH):
                    w1dma(m_)
                nc.sync.dma_start(vbt[:], vb[:])
                nc.sync.dma_start(wc2t[:], wc2[:])
            rep_ctx = tc.For_i(0, repeat, 1) if repeat > 1 else nullcontext()
            with rep_ctx:
                _kernel_body(nc, mybir, nbt, st_dt, xpool, spool, hpool, opool,
                             ps_blk, ps_srep, ps_pre, ps_out, yT, consts,
                             first=(repeat == 1))

    nc.compile()
    return nc


def _kernel_body(nc, mybir, nbt, st_dt, xpool, spool, hpool, opool,
                 ps_blk, ps_srep, ps_pre, ps_out, yT, consts, first):
    AF = mybir.ActivationFunctionType
    OP = mybir.AluOpType
    fp32 = mybir.dt.float32
    w1blk = consts["w1blk"]
    bct = consts["bct"]
    xdma = consts["xdma"]
    w1dma = consts["w1dma"]

    xk = [None] * nbt
    if first:
        # Startup choreography: the gate block needs only the gate weights
        # plus progressive x0 quarters; interleave so nothing queues behind
        # bulk transfers it doesn't need yet, and pull w1_m0 ahead of the
        # late x0 quarters (layer-1 m0 starts ~1.7us after the gate).
        bct_t, bcol_d = consts["bctd"]
        w1dma(MH)
        xt0 = xpool.tile([128, KD * NTILE], st_dt, tag="x")
        Q = KD * NTILE // 4
        nc.sync.dma_start(xt0[:, 0:Q], consts["xsrc"](0, 0, Q))
        nc.sync.dma_start(xt0[:, Q : 2 * Q], consts["xsrc"](0, Q, 2 * Q))
        w1dma(0)
        nc.sync.dma_start(bct_t[:], bcol_d[:])
        nc.sync.dma_start(xt0[:, 2 * Q : 3 * Q], consts["xsrc"](0, 2 * Q, 3 * Q))
        nc.sync.dma_start(xt0[:, 3 * Q : 4 * Q], consts["xsrc"](0, 3 * Q, 4 * Q))
        xk[0] = xt0
        for m_ in range(1, MH):
            w1dma(m_)
        xk[1] = xdma(1)
        nc.sync.dma_start(consts["vbt"][:], consts["vb"][:])
        nc.sync.dma_start(consts["wc2t"][:], consts["wc2"][:])
    else:
        xk[0] = xdma(0)
        xk[1] = xdma(1)

    def vblk(k):
        return consts["vbt"][:, k * 128 : (k + 1) * 128]

    def emit_vfold(st):
        """PE: 8 padded-V matmuls + Cm-tile matmul -> ps_pre; ACT relu -> rp.

        All K=128, M=128: rows 64:127 of pre are exact zeros (zero-padded V
        columns), relu'd with zero bias into rp rows 64:127 so rp can feed
        the padded K=128 cls2 matmul with no masking.
        """
        pp = ps_pre.tile([128, NTILE], fp32, tag="pre")
        for k in range(MH):
            nc.tensor.matmul(
                pp[:], vblk(k), st["hs"][k][:], start=(k == 0), stop=False
            )
        nc.tensor.matmul(pp[:], vblk(MH), st["gw"][:], start=False, stop=True)
        rp = spool.tile([128, NTILE], st_dt, tag="rp")
        nc.scalar.activation(rp[:], pp[:], AF.Relu, bias=bct[:, 9:10])
        st["rp"] = rp

    def emit_cls2(st):
        op_ = ps_out.tile([128, NTILE], fp32, tag="out")
        nc.tensor.matmul(
            op_[:], consts["wc2t"][:], st["rp"][:], start=True, stop=True
        )
        ot = opool.tile([NCLS, NTILE], fp32, tag="o")
        nc.scalar.activation(
            ot[:], op_[0:NCLS, :], AF.Identity, bias=bct[0:NCLS, 10:11]
        )
        b0 = st["t"] * NTILE
        nc.sync.dma_start(yT[0:NCLS, b0 : b0 + NTILE], ot[:])

    p1 = None  # tile t-1 state (awaiting V-fold)
    p2 = None  # tile t-2 state (awaiting cls2)
    for t in range(nbt):
        if t + 2 < nbt:
            xk[t + 2] = xdma(t + 2)

        # ---- gate block: replicated logits; exp(logit + bg) on ACT ----
        gp = ps_blk.tile([128, NTILE], fp32, tag="blk")
        for k in range(KD):
            nc.tensor.matmul(
                gp[:], w1blk(MH, k), xk[t][:, k * NTILE : (k + 1) * NTILE],
                start=(k == 0), stop=(k == KD - 1),
            )
        expg = spool.tile([128, NTILE], st_dt, tag="expg")
        nc.scalar.activation(expg[:], gp[:], AF.Exp, bias=bct[:, 8:9])

        # ---- layer-1 h-blocks: relu(.+be1) on ACT; gate*h on DVE ----
        # The softmax denom matmul slots between h-blocks 0 and 1: by then
        # the ACT exp is long done (no PE stall), and the DVE chain
        # (1/s -> gw -> 8 gate mults) overlaps h-blocks 1..7, so hs(t) is
        # complete right when the PE reaches V-fold(t) one tile later.
        # For the FINAL tile the V-fold matmuls interleave directly into the
        # layer-1 stream (disjoint PSUM banks, lag 2 blocks behind the hs
        # chain) so the epilogue exposes only the last cls chain.
        last = t == nbt - 1
        pp_last = (
            ps_pre.tile([128, NTILE], fp32, tag="pre", name="pp_last")
            if last
            else None
        )
        hs = []
        gw = None
        for m in range(MH):
            hp = ps_blk.tile([128, NTILE], fp32, tag="blk")
            for k in range(KD):
                nc.tensor.matmul(
                    hp[:], w1blk(m, k), xk[t][:, k * NTILE : (k + 1) * NTILE],
                    start=(k == 0), stop=(k == KD - 1),
                )
            hr = hpool.tile([128, NTILE], st_dt, tag=f"hs{m}")
            nc.scalar.activation(hr[:], hp[:], AF.Relu, bias=bct[:, m : m + 1])
            hs.append(hr)
            if m == 0:
                sp = ps_srep.tile([128, NTILE], fp32, tag="srep")
                nc.tensor.matmul(
                    sp[:], consts["ones16"][:], expg[:], start=True, stop=True
                )
                rinv = spool.tile([128, NTILE], fp32, tag="rinv")
                nc.vector.reciprocal(rinv[:], sp[:])
                gw = spool.tile([128, NTILE], st_dt, tag="gw")
                nc.vector.tensor_tensor(gw[:], expg[:], rinv[:], op=OP.mult)
            nc.vector.tensor_tensor(hs[m][:], hs[m][:], gw[:], op=OP.mult)
            if last and m >= 2:
                nc.tensor.matmul(
                    pp_last[:], vblk(m - 2), hs[m - 2][:],
                    start=(m == 2), stop=False, skip_group_check=True,
                )
        xk[t] = None

        # ---- pipelined tails: V-fold(t-1) then cls2(t-2) on PE ----
        if p1 is not None:
            emit_vfold(p1)
        if p2 is not None:
            emit_cls2(p2)
        p2 = p1
        p1 = {"t": t, "hs": hs, "gw": gw}

    # epilogue: finish the last tile's interleaved V-fold, then drain the
    # cls chain in column halves so ACT/PE/DMA pipeline instead of
    # serializing on the full-width pre -> rp -> cls2 -> out chain.
    HN = NTILE // 2
    for k in range(MH - 2, MH):
        nc.tensor.matmul(
            pp_last[:], vblk(k), hs[k][:],
            start=False, stop=False, skip_group_check=True,
        )
    for h_ in range(2):
        nc.tensor.matmul(
            pp_last[:, h_ * HN : (h_ + 1) * HN], vblk(MH),
            gw[:, h_ * HN : (h_ + 1) * HN],
            start=False, stop=True, skip_group_check=True,
        )
    emit_cls2(p2)  # fills the PE gap while rp half 0 is on ACT
    rp = spool.tile([128, NTILE], st_dt, tag="rp")
    op_ = ps_out.tile([128, NTILE], fp32, tag="out", name="op_last")
    ot = opool.tile([NCLS, NTILE], fp32, tag="o", name="ot_last")
    b0 = p1["t"] * NTILE
    for h_ in range(2):
        sl = slice(h_ * HN, (h_ + 1) * HN)
        nc.scalar.activation(
            rp[:, sl], pp_last[:, sl], AF.Relu, bias=bct[:, 9:10]
        )
        nc.tensor.matmul(
            op_[:, sl], consts["wc2t"][:], rp[:, sl],
            start=True, stop=True, skip_group_check=True,
        )
        nc.scalar.activation(
            ot[:, sl], op_[0:NCLS, sl], AF.Identity, bias=bct[0:NCLS, 10:11]
        )
        nc.sync.dma_start(
            yT[0:NCLS, b0 + h_ * HN : b0 + (h_ + 1) * HN], ot[:, sl]
        )


def _get_nc(b_per_core: int, mm_dt_name: str, repeat: int = 1):
    key = (b_per_core, mm_dt_name, repeat)
    if key not in _BUILT:
        _BUILT[key] = _build_nc(b_per_core, mm_dt_name, repeat)
    return _BUILT[key]


def prep_inputs(x, We1, be1, We2, be2, Wg, bg, Wc1, bc1, Wc2, bc2,
                mm_dt_name=MM_DT, n_cores=NCORES):
    """Host-side packing -> list of per-core input maps."""
    f64 = np.float64
    sdt = _np_store_dt(mm_dt_name)
    b_per_core = x.shape[0] // n_cores

    # feature order f = j*E + e
    W1_all = np.transpose(np.asarray(We1, f64), (1, 2, 0)).reshape(D, F)
    Wg_rep = np.asarray(Wg, f64)[:, np.arange(128) % E]
    blocks = []
    for m_ in range(MH):
        for k in range(KD):
            blocks.append(W1_all[k * 128 : (k + 1) * 128, m_ * 128 : (m_ + 1) * 128])
    for k in range(KD):
        blocks.append(Wg_rep[k * 128 : (k + 1) * 128, :])
    W1T = np.ascontiguousarray(np.concatenate(blocks, axis=1).astype(sdt))

    # V blocks zero-padded to M=128; final block is tile(Cm)/16, contracted
    # against the fully-replicated gw (16 copies x 1/16 = exact sum over E).
    V = np.einsum("ejk,kc->jec", np.asarray(We2, f64), np.asarray(Wc1, f64)).reshape(
        F, C1
    )
    Cm = np.asarray(be2, f64) @ np.asarray(Wc1, f64)  # [E, C1]
    vb_blocks = []
    for k in range(MH):
        blk = np.zeros((128, 128), f64)
        blk[:, 0:C1] = V[k * 128 : (k + 1) * 128, :]
        vb_blocks.append(blk)
    dblk = np.zeros((128, 128), f64)
    dblk[:, 0:C1] = Cm[np.arange(128) % E, :] / 16.0
    vb_blocks.append(dblk)
    Vb = np.ascontiguousarray(np.concatenate(vb_blocks, axis=1).astype(sdt))
    WC2 = np.zeros((128, 128), f64)
    WC2[0:C1, 0:NCLS] = np.asarray(Wc2, f64)
    WC2 = np.ascontiguousarray(WC2.astype(sdt))

    bcol = np.zeros((128, 11), np.float32)
    be1_int = np.asarray(be1, f64).T.reshape(F)  # f = j*E + e
    for m_ in range(MH):
        bcol[:, m_] = be1_int[m_ * 128 : (m_ + 1) * 128]
    bcol[:, 8] = np.asarray(bg, f64)[np.arange(128) % E]
    bcol[0:C1, 9] = np.asarray(bc1, f64)
    bcol[0:NCLS, 10] = np.asarray(bc2, f64)

    # pack x tile-major: xP[p, t, k, c] = x[t*NTILE+c, k*128+p] per core, so
    # each 512-row btile is one contiguous-per-partition 8KB DMA.
    nbt = b_per_core // NTILE
    xT_full = np.asarray(x).T.astype(sdt)  # [D, B]
    in_maps = []
    for c in range(n_cores):
        xc = xT_full[:, c * b_per_core : (c + 1) * b_per_core]
        xP = np.ascontiguousarray(
            xc.reshape(KD, 128, nbt, NTILE)
            .transpose(1, 2, 0, 3)
            .reshape(128, nbt * KD * NTILE)
        )
        in_maps.append(
            {
                "xT": xP,
                "W1T": W1T,
                "Vb": Vb,
                "WC2": WC2,
                "BCOL": bcol,
            }
        )
    return in_maps, b_per_core


def run(inputs, mm_dt_name=MM_DT, trace=False):
    """Run on 8 NeuronCores; returns (y [B, 2] fp32, exec_time_ns or None)."""
    from concourse.bass_utils import run_bass_kernel_spmd

    in_maps, b_per_core = prep_inputs(**inputs, mm_dt_name=mm_dt_name)
    nc = _get_nc(b_per_core, mm_dt_name)
    res = run_bass_kernel_spmd(
        nc, in_maps, core_ids=list(range(NCORES)), trace=trace
    )
    y = np.concatenate([r["yT"].T for r in res.results], axis=0)
    return np.ascontiguousarray(y.astype(np.float32)), res.exec_time_ns


def kernel(**inputs):
    y, _ = run(inputs)
    return y
